# revision 51
# baseline (speedup 1.0000x reference)
"""Trainium2 Bass kernel for nn_AspectModel (span-attention aspect tagger).

Strategy: batch-shard the 32 sentences 4-per-core across 8 NeuronCores; route
each fragment (host-side) to the core owning its sentence, padded to 64 slots
per sentence (256 slots/core).  All heavy math runs on-chip:
  - span features (l_word / word_state / r_word) via a masks-matmul against
    the sentence hidden states (one-hot + in-span masks built on-chip),
  - v = span @ att_w and c = span @ att_b as dense matmuls over all slots,
  - attention scores via a PE matmul of V against the transposed memory
    (transpose done by the DMA xbar in bf16),
  - masked softmax (fused exp+sum) and mix via a second masks-matmul,
  - tag logits + log_softmax.
Matmul operands are cast to bf16 on-chip (f32 PSUM accumulation); the f32
tensor-engine path costs 2 passes per matmul, bf16 costs 1.
Each core returns its own [256, 5] slot outputs; the host scatters them back
into the full [1024, 5] output.  No collectives needed.
"""

import sys
import types

import ml_dtypes
import numpy as np

# Optional shim so run_bass_kernel_spmd(trace=True) works in containers where
# antenv.axon_hooks is missing (profiling only; correctness path unaffected).
try:
    import antenv.axon_hooks  # noqa: F401
except ImportError:
    try:
        from trn_agent_boot.trn_boot import _ntff_profile_via_ctypes

        _hook = _ntff_profile_via_ctypes("/opt/axon/libaxon_pjrt.so")
        _mod = types.ModuleType("antenv.axon_hooks")
        _mod.get_axon_ntff_profile_hook = lambda: _hook
        _mod.set_axon_ntff_profile_hook = lambda h: None
        sys.modules["antenv.axon_hooks"] = _mod
    except Exception:
        pass

import concourse.bass as bass  # noqa: E402
import concourse.tile as tile  # noqa: E402
from concourse import bacc, mybir  # noqa: E402
from concourse import bass_utils  # noqa: E402
from concourse.bass_utils import run_bass_kernel_spmd  # noqa: E402

# No artifact bucket in the sandbox; make tracing's upload step a no-op.
bass_utils.upload_artifacts = lambda tmpdir: f"local:{tmpdir}"

F32 = mybir.dt.float32
BF16 = mybir.dt.bfloat16
I32 = mybir.dt.int32
ALU = mybir.AluOpType
ACT = mybir.ActivationFunctionType

B, S, D, F, T = 32, 256, 512, 1024, 5
NCORES = 8
SEN = 4          # sentences per core
G = 64           # fragment slots per sentence
C = SEN * G      # 256 fragment slots per core
D3 = 3 * D

TRACE = False
LAST_RESULT = None  # BassKernelResults of the most recent run (for test.py)

# Build-time knobs (for A/B experiments; defaults are the shipped config).
OPTS = {
    "memt_eng": "sync",     # engine issuing the memT xbar transposes
    "tiny_eng": "sync",     # engine issuing the small input DMAs
    "mk_first": True,       # emit masksT DVE ops before the bf16 casts
    "ln_dep": True,         # order Ln after both Exp (ACT table thrash)
}

_compiled = {}


def _build(seq_len: float):
    """Build + compile the per-core SPMD graph (identical on all 8 cores)."""
    nc = bacc.Bacc("TRN2", target_bir_lowering=False, debug=False,
                   num_devices=NCORES)

    # All inputs are laid out host-side as [partition, free] so every DMA is a
    # dense per-partition contiguous read.
    x_d = nc.dram_tensor("x", [128, 2 * SEN, D], F32, kind="ExternalInput")
    aw_d = nc.dram_tensor("aw", [128, 12, D], F32, kind="ExternalInput")
    # awt = [ab (12 cols) | tag_w.T packed (16*5 cols)]
    awt_d = nc.dram_tensor("awt", [128, 12 + 16 * T], F32,
                           kind="ExternalInput")
    # meta1 cols: 0-1 fs_c, 2-3 fm_c, 4-5 ln_c; row 0 cols 6-10: tag_b
    meta1_d = nc.dram_tensor("meta1", [128, 12], F32, kind="ExternalInput")
    # host-built span masks, [s%128, s//128, comp, slot] (bf16 0/1)
    mkT_d = nc.dram_tensor("mkT", [128, 2, 4, C], BF16, kind="ExternalInput")
    out_d = nc.dram_tensor("out", [128, 2, T], F32, kind="ExternalOutput")

    with tile.TileContext(nc) as tc:
        with (
            tc.tile_pool(name="persist", bufs=1) as pp,
            tc.tile_pool(name="work", bufs=2) as wp,
            tc.tile_pool(name="psum", bufs=2, space="PSUM") as psp,
        ):
            # ---- persistent SBUF tensors ----
            x_sb = pp.tile([128, 2 * SEN, D], F32, tag="x_sb")
            x_bf = pp.tile([128, 2 * SEN, D], BF16, tag="x_bf")
            aw_sb = pp.tile([128, 12, D], F32, tag="aw_sb")
            aw_bf = pp.tile([128, 12, D], BF16, tag="aw_bf")
            awt_sb = pp.tile([128, 12 + 16 * T], F32, tag="awt_sb")
            awt_bf = pp.tile([128, 12 + 16 * T], BF16, tag="awt_bf")
            meta1 = pp.tile([128, 12], F32, tag="meta1")
            tbb = pp.tile([128, T], F32, tag="tbb")
            iota_i = pp.tile([128, S], I32, tag="iota_i")
            iota_f = pp.tile([128, S], F32, tag="iota_f")
            iota_n = pp.tile([128, S], F32, tag="iota_n")
            mkT_sb = pp.tile([128, 2, 4, C], BF16, tag="mkT_sb")
            mkT = [mkT_sb[:, k, 0:3, :] for k in range(2)]
            spanT = pp.tile([128, 12, C], BF16, tag="spanT")
            v_sb = pp.tile([128, 4, C], BF16, tag="v_sb")
            memT = pp.tile([128, SEN, 2, 4, 128], BF16, tag="memT")
            mixT = pp.tile([128, 4, C], BF16, tag="mixT")

            memt_eng = nc.sync if OPTS["memt_eng"] == "sync" else nc.scalar

            # ---- input DMAs (x first on sync; small stuff on `tiny`) ----
            tiny = {"sync": nc.sync, "scalar": nc.scalar,
                    "gpsimd": nc.gpsimd}[OPTS["tiny_eng"]]
            m1_dma = tiny.dma_start(meta1[:], meta1_d.ap())
            m2_dma = tiny.dma_start(mkT_sb[:], mkT_d.ap())
            nc.scalar.dma_start(awt_sb[:], awt_d.ap())
            prev = None
            for half in range(2):
                xd = nc.sync.dma_start(
                    x_sb[:, half * 4:(half + 1) * 4, :],
                    x_d.ap()[:, half * 4:(half + 1) * 4, :])
                # pin ring order: metas, then x halves, then aw thirds
                tile.add_dep_helper(xd.ins, m1_dma.ins, sync=False,
                                    reason="metas before x")
                tile.add_dep_helper(xd.ins, m2_dma.ins, sync=False,
                                    reason="metas before x")
                if prev is not None:
                    tile.add_dep_helper(xd.ins, prev.ins, sync=False,
                                        reason="x order")
                prev = xd
            for third in range(3):
                ad = nc.sync.dma_start(
                    aw_sb[:, third * 4:(third + 1) * 4, :],
                    aw_d.ap()[:, third * 4:(third + 1) * 4, :])
                tile.add_dep_helper(ad.ins, prev.ins, sync=False,
                                    reason="aw after x")
                prev = ad

            fs_c = meta1[:, 0:2]
            fm_c = meta1[:, 2:4]
            ln_c = meta1[:, 4:6]
            tb_sb = meta1[0:1, 6:6 + T]

            # ---- constants ----
            neg4 = pp.tile([128, 1], F32, tag="neg4")
            nc.gpsimd.memset(neg4[:], -1.0e4)
            nc.gpsimd.iota(iota_i[:], pattern=[[1, S]], channel_multiplier=0)
            nc.vector.tensor_copy(iota_f[:], iota_i[:])
            nc.vector.tensor_scalar_mul(iota_n[:], iota_f[:], -1.0)
            nc.gpsimd.partition_broadcast(tbb[:], tb_sb)

            def emit_casts():
                for half in range(2):
                    nc.vector.tensor_copy(x_bf[:, half * 4:(half + 1) * 4, :],
                                          x_sb[:, half * 4:(half + 1) * 4, :])
                nc.any.tensor_copy(awt_bf[:], awt_sb[:])
                for third in range(3):
                    nc.any.tensor_copy(
                        aw_bf[:, third * 4:(third + 1) * 4, :],
                        aw_sb[:, third * 4:(third + 1) * 4, :])

            emit_casts()

            # ---- memT[d, s] via DMA xbar transpose (bf16) ----
            # One big transpose per 2-sentence half: [128, 2048] -> logical
            # [2048, 128]; row r = (l*2+k)*512 + d lands at chunk
            # m = (l*2+k)*4 + dj, partition d%128.  memT is [128, l, k, dj, s'].
            for hf in range(2):
                memt_eng.dma_start_transpose(
                    memT[:, 2 * hf:2 * hf + 2, :, :, :],
                    x_bf[:, hf * 4:(hf + 1) * 4, :])

            # ---- span masks-matmul: spanT[3D, C] (l_word | word_state | r_word)
            sc_span = nc.named_scope("spanmm"); sc_span.__enter__()
            for l in range(SEN):
                for j0 in range(2):  # pairs of D chunks of the word vectors
                    ps = psp.tile([128, 2, 3, G], F32, tag="psm")
                    for dj in range(2):
                        j = j0 * 2 + dj
                        for k in range(2):
                            nc.tensor.matmul(
                                ps[:, dj, :, :],
                                x_bf[:, l * 2 + k, j * 128:(j + 1) * 128],
                                mkT[k][:, :, l * G:(l + 1) * G],
                                start=(k == 0), stop=(k == 1),
                            )
                    nc.any.tensor_copy(
                        spanT[:, j0 * 6:j0 * 6 + 6, l * G:(l + 1) * G], ps[:])

            sc_span.__exit__(None, None, None)
            # ---- v = span @ att_w  (stored transposed: V[d, slot])
            sc_v = nc.named_scope("vmm"); sc_v.__enter__()
            for m0 in range(2):
                pv = psp.tile([128, 2, C], F32, tag="p2k")
                for mj in range(2):
                    m = m0 * 2 + mj
                    for kk in range(12):
                        nc.tensor.matmul(
                            pv[:, mj, :],
                            aw_bf[:, kk, m * 128:(m + 1) * 128],
                            spanT[:, kk, :],
                            start=(kk == 0), stop=(kk == 11),
                        )
                nc.any.tensor_copy(v_sb[:, m0 * 2:m0 * 2 + 2, :], pv[:])

            sc_v.__exit__(None, None, None)
            # ---- c = span @ att_b as a row vector, then partition-broadcast
            sc_c = nc.named_scope("cmm"); sc_c.__enter__()
            pc = psp.tile([1, C], F32, tag="pout", bufs=4)
            for kk in range(12):
                nc.tensor.matmul(
                    pc[:],
                    awt_bf[:, kk:kk + 1],
                    spanT[:, kk, :],
                    start=(kk == 0), stop=(kk == 11),
                )
            c_row = pp.tile([1, C], F32, tag="c_row")
            nc.any.tensor_copy(c_row[:], pc[:])
            c_bc = pp.tile([128, C], F32, tag="c_bc")
            nc.gpsimd.partition_broadcast(c_bc[:], c_row[:])

            sc_c.__exit__(None, None, None)
            # ---- pos-weight (pw) per pair, then transpose to [s, slot] ----
            ones_bf = pp.tile([128, 1], BF16, tag="ones_bf")
            nc.gpsimd.memset(ones_bf[:], 1.0)
            pwT = pp.tile([128, 2, 2, 128], BF16, tag="pwT")  # [s', q, k, slot]
            for q in range(2):
                fs_q = fs_c[:, q:q + 1]
                fm_q = fm_c[:, q:q + 1]
                t1 = wp.tile([128, S], F32, tag="t1")
                t2 = wp.tile([128, S], F32, tag="t2")
                dm = wp.tile([128, S], F32, tag="dm")
                pwr = wp.tile([128, S], F32, tag="pwr")
                noti = wp.tile([128, S], F32, tag="noti")
                pwb = wp.tile([128, S], BF16, tag="pwb")
                nc.vector.tensor_scalar(t1[:], iota_n[:], fs_q, None,
                                        op0=ALU.add)            # fs - s
                nc.vector.tensor_scalar(t2[:], iota_f[:], fm_q, None,
                                        op0=ALU.subtract)       # s - (fe-1)
                nc.vector.tensor_tensor(dm[:], t1[:], t2[:], op=ALU.max)
                nc.vector.tensor_scalar(pwr[:], dm[:], -1.0 / seq_len, 1.0,
                                        op0=ALU.mult, op1=ALU.add)
                nc.vector.tensor_single_scalar(noti[:], dm[:], 0.0,
                                               op=ALU.is_gt)    # not in span
                nc.vector.tensor_tensor(pwb[:], pwr[:], noti[:], op=ALU.mult)
                # [slot, s] -> [s', k, slot]
                nc.sync.dma_start_transpose(pwT[:, q, :, :], pwb[:])

            # ---- per sentence-pair attention + mix + logits ----
            # scores kept transposed ([s, slot]); softmax denominator via a
            # PE ones-matmul; normalization folded into the logits scale.
            sh_t = pp.tile([128, 2, T], F32, tag="sh_t")
            se_t = pp.tile([128, 2], F32, tag="se_t")
            res_t = pp.tile([128, 2, T], F32, tag="res_t")

            # PE-early: both pairs' score matmuls and span-logits, so the PE
            # FIFO never stalls behind the pair-0 activation chain.
            gts = []
            for q in range(2):
                gt = psp.tile([128, 2, 128], F32, tag="p2k", name=f"gt{q}")
                gts.append(gt)
                for h in range(2):
                    l = 2 * q + h
                    for k in range(2):
                        for dj in range(4):
                            nc.tensor.matmul(
                                gt[:, k, h * G:(h + 1) * G],
                                memT[:, l, k, dj, :],
                                v_sb[:, dj, l * G:(l + 1) * G],
                                start=(dj == 0), stop=(dj == 3),
                            )
            plss = []
            for q in range(2):
                pls = psp.tile([128, T], F32, tag="pout", bufs=4, name=f"pls{q}")
                plss.append(pls)
                for kk in range(12):
                    nc.tensor.matmul(
                        pls[:], spanT[:, kk, q * 128:(q + 1) * 128],
                        awt_bf[:, 12 + kk * T:12 + (kk + 1) * T],
                        start=(kk == 0), stop=(kk == 11))

            ex2_insts = []
            for q in range(2):
                gt = gts[q]
                sg = wp.tile([128, 2, 128], F32, tag="sg")
                th = wp.tile([128, 2, 128], F32, tag="th")
                thm = wp.tile([128, 2, 128], F32, tag="thm")
                uT = wp.tile([128, 2, 128], BF16, tag="uT")
                wTu = wp.tile([128, 2, 128], BF16, tag="wTu")
                rden = wp.tile([128, 1], F32, tag="rden")

                # scores = tanh(pw * G + c); masked exp (still un-normalized)
                nc.vector.tensor_tensor(sg[:], gt[:], pwT[:, q, :, :],
                                        op=ALU.mult)
                cb = c_bc[:, q * 128:(q + 1) * 128]
                nc.vector.tensor_tensor(
                    sg[:], sg[:],
                    cb.rearrange("p (o c) -> p o c",
                                 o=1).broadcast_to([128, 2, 128]),
                    op=ALU.add)
                nc.scalar.activation(th[:], sg[:], ACT.Tanh)
                kpq = mkT_sb[:, :, 3, q * 128:(q + 1) * 128]
                nc.vector.scalar_tensor_tensor(thm[:], th[:], 1.0e4, kpq,
                                               op0=ALU.add, op1=ALU.mult)
                nc.scalar.activation(uT[:], thm[:], ACT.Exp, bias=neg4[:])

                # denominator via ones-matmul ([slot, 1] per pair)
                dn = psp.tile([128, 1], F32, tag="pout", bufs=4)
                for k in range(2):
                    nc.tensor.matmul(dn[:], uT[:, k, :], ones_bf[:],
                                     start=(k == 0), stop=(k == 1))
                nc.vector.reciprocal(rden[:], dn[:])
                nc.vector.tensor_tensor(wTu[:], uT[:], pwT[:, q, :, :],
                                        op=ALU.mult)

                # mixT_unnorm[d, slot] = sum_s mem[s, d] * u[slot, s] * pw
                for h in range(2):
                    l = 2 * q + h
                    pm = psp.tile([128, 4, G], F32, tag="psm")
                    for dj in range(4):
                        for k in range(2):
                            nc.tensor.matmul(
                                pm[:, dj, :],
                                x_bf[:, l * 2 + k, dj * 128:(dj + 1) * 128],
                                wTu[:, k, h * G:(h + 1) * G],
                                start=(k == 0), stop=(k == 1),
                            )
                    nc.any.tensor_copy(mixT[:, :, l * G:(l + 1) * G], pm[:])

                plm = psp.tile([128, T], F32, tag="pout", bufs=4)
                for dj in range(4):
                    nc.tensor.matmul(
                        plm[:], mixT[:, dj, q * 128:(q + 1) * 128],
                        awt_bf[:, 12 + (12 + dj) * T:12 + (13 + dj) * T],
                        start=(dj == 0), stop=(dj == 3))

                # logits = pls + rden*plm + tb; log-softmax without the max
                # shift (logits are small; exp is safe in f32)
                lg = wp.tile([128, T], F32, tag="lg")
                nc.vector.scalar_tensor_tensor(lg[:], plm[:], rden[:], tbb[:],
                                               op0=ALU.mult, op1=ALU.add)
                nc.vector.tensor_tensor(sh_t[:, q, :], lg[:], plss[q][:],
                                        op=ALU.add)
                ex2 = wp.tile([128, T], F32, tag="ex2")
                ex2_insts.append(
                    nc.scalar.activation(ex2[:], sh_t[:, q, :], ACT.Exp,
                                         accum_out=se_t[:, q:q + 1]))

            # ---- log-softmax epilogue: one Ln + one subtract for both
            # pairs (reads of se_t[:, :] order it after both Exps natively)
            lse2 = wp.tile([128, 2], F32, tag="lse2")
            nc.scalar.activation(lse2[:], se_t[:], ACT.Ln)
            nc.vector.tensor_tensor(
                res_t[:], sh_t[:],
                lse2.rearrange("p (q o) -> p q o", o=1).broadcast_to(
                    [128, 2, T]),
                op=ALU.subtract)
            nc.sync.dma_start(out_d.ap(), res_t[:])

    nc.compile()
    return nc


def _host_prep(en_output, lengths, frag_b, frag_s, frag_e, att_w, att_b,
               tag_w, tag_b):
    """Shard + relayout inputs.  Returns (in_maps, assign, overflow)."""
    # replicated weights, permuted so spanT chunk kk = 3*j + comp maps to
    # att rows comp*512 + j*128 : .. + 128.
    perm = np.concatenate([
        np.arange(comp * D + j * 128, comp * D + (j + 1) * 128)
        for j in range(4) for comp in range(3)
    ])
    aw_np = att_w[perm].reshape(12, 128, D).transpose(1, 0, 2).reshape(128, 12, D)
    ab_np = att_b[perm].reshape(12, 128).T.copy()
    tw_rows = np.concatenate([tag_w[:, perm].T,
                              tag_w[:, D3:].T], axis=0)  # [2048, 5]
    tw_np = tw_rows.reshape(16, 128, T).transpose(1, 0, 2).reshape(128, 16, T)
    tb_np = tag_b.reshape(1, T)

    aw_np = np.ascontiguousarray(aw_np, dtype=np.float32)
    ab_np = np.ascontiguousarray(ab_np, dtype=np.float32)
    tw_np = np.ascontiguousarray(tw_np, dtype=np.float32)
    tb_np = np.ascontiguousarray(tb_np, dtype=np.float32)

    assign = np.full((F, 2), -1, dtype=np.int64)  # (core, slot) per fragment
    counts = np.zeros((NCORES, SEN), dtype=np.int64)
    overflow = []
    in_maps = []

    fs_slot = np.zeros((NCORES, C), np.float32)
    fm_slot = np.zeros((NCORES, C), np.float32)
    ln_slot = np.full((NCORES, C), float(S), np.float32)

    for i in range(F):
        b = int(frag_b[i])
        core, l = b // SEN, b % SEN
        k = counts[core, l]
        if k >= G:
            overflow.append(i)
            continue
        counts[core, l] += 1
        slot = l * G + k
        assign[i] = (core, slot)
        fs_slot[core, slot] = frag_s[i]
        fm_slot[core, slot] = frag_e[i] - 1
        ln_slot[core, slot] = lengths[b]

    awt_np = np.concatenate(
        [ab_np, tw_np.reshape(128, 16 * T)], axis=1).astype(np.float32)
    awt_np = np.ascontiguousarray(awt_np)

    for core in range(NCORES):
        xs = en_output[core * SEN:(core + 1) * SEN]  # [4, 256, 512]
        x_np = np.ascontiguousarray(
            xs.reshape(SEN, 2, 128, D).transpose(2, 0, 1, 3)
              .reshape(128, 2 * SEN, D), dtype=np.float32)
        meta1 = np.zeros((128, 12), np.float32)
        meta1[:, 0:2] = fs_slot[core].reshape(2, 128).T
        meta1[:, 2:4] = fm_slot[core].reshape(2, 128).T
        meta1[:, 4:6] = ln_slot[core].reshape(2, 128).T
        meta1[0, 6:6 + T] = tb_np[0]
        # span masks [S, 4, C] -> [128, 2, 4, C] (s = k*128 + p);
        # component 3 is the attention keep-mask (!in_span & s < len)
        pos = np.arange(S, dtype=np.float32)[:, None]
        fs = fs_slot[core][None, :]
        fm = fm_slot[core][None, :]
        ln = ln_slot[core][None, :]
        mk = np.empty((S, 4, C), np.float32)
        in_span = (pos >= fs) & (pos <= fm)
        mk[:, 0, :] = pos == fs
        mk[:, 1, :] = in_span
        mk[:, 2, :] = pos == fm
        mk[:, 3, :] = (~in_span) & (pos < ln)
        mk = np.ascontiguousarray(
            mk.reshape(2, 128, 4, C).transpose(1, 0, 2, 3)).astype(
                ml_dtypes.bfloat16)
        in_maps.append({
            "x": x_np, "aw": aw_np, "awt": awt_np,
            "meta1": meta1, "mkT": mk,
        })
    return in_maps, assign, overflow


def _host_fragment(en_output, lengths, s, e, b, att_w, att_b, tag_w, tag_b,
                   seq_len):
    """Numpy fallback for (vanishingly rare) slot-overflow fragments."""
    mem = en_output[b].astype(np.float64)
    ws = mem[s:e].sum(0)
    span = np.concatenate([mem[s], ws, mem[e - 1]])
    pos = np.arange(S)
    in_span = (pos >= s) & (pos < e)
    att_mask = in_span | (pos >= lengths[b])
    dis = np.where(pos < s, s - pos,
                   np.where(pos >= e, pos - e + 1, seq_len)).astype(np.float64)
    pwv = 1.0 - dis / seq_len
    fin = pwv[:, None] * mem
    v = span @ att_w.astype(np.float64)
    c = span @ att_b.astype(np.float64)
    sc = np.tanh(fin @ v + c)
    sc = np.where(att_mask, -1e4, sc)
    sc = sc - sc.max()
    a = np.exp(sc)
    a = a / a.sum()
    mix = a @ fin
    ms = np.concatenate([span, mix])
    lg = ms @ tag_w.astype(np.float64).T + tag_b.astype(np.float64)
    lg = lg - lg.max()
    return (lg - np.log(np.exp(lg).sum())).astype(np.float32)


def kernel(en_output, lengths, frag_b, frag_s, frag_e, att_w, att_b, tag_w,
           tag_b):
    global LAST_RESULT
    en_output = np.asarray(en_output, dtype=np.float32)
    lengths = np.asarray(lengths).astype(np.int64)
    frag_b = np.asarray(frag_b).astype(np.int64)
    frag_s = np.asarray(frag_s).astype(np.int64)
    frag_e = np.asarray(frag_e).astype(np.int64)
    att_w = np.asarray(att_w, dtype=np.float32)
    att_b = np.asarray(att_b, dtype=np.float32)
    tag_w = np.asarray(tag_w, dtype=np.float32)
    tag_b = np.asarray(tag_b, dtype=np.float32)

    seq_len = float(lengths[0])
    key = (seq_len, tuple(sorted(OPTS.items())))
    if key not in _compiled:
        _compiled[key] = _build(seq_len)
    nc = _compiled[key]

    in_maps, assign, overflow = _host_prep(
        en_output, lengths, frag_b, frag_s, frag_e, att_w, att_b, tag_w, tag_b)

    res = run_bass_kernel_spmd(nc, in_maps, core_ids=list(range(NCORES)),
                               trace=TRACE)
    LAST_RESULT = res

    out = np.empty((F, T), dtype=np.float32)
    per_core = [res.results[i]["out"].transpose(1, 0, 2).reshape(C, T)
                for i in range(NCORES)]
    cores = assign[:, 0]
    slots = assign[:, 1]
    for core in range(NCORES):
        sel = cores == core
        out[sel] = per_core[core][slots[sel]]
    for i in overflow:
        out[i] = _host_fragment(en_output, lengths, int(frag_s[i]),
                                int(frag_e[i]), int(frag_b[i]), att_w, att_b,
                                tag_w, tag_b, seq_len)
    return out
